# revision 49
# baseline (speedup 1.0000x reference)
"""Trainium2 Bass kernel for multi-head self-attention (B=2, N=2048, DIM=1024,
16 heads x 64). Sharding: core i handles batch b=i//4 and 4 heads hg=i%4
(tensor-parallel on heads: column-shard Wq/Wkv, row-shard Wo; partial outputs
summed on host).

Math notes:
  - `similarity` adds a per-query constant along the softmax axis, so softmax
    is invariant to it -> it is accepted but unused.
  - Softmax computed without max-subtraction (logits are O(10); exp is safe in
    fp32) as exp(dots)/Z with Z obtained for free as a 65th "ones" column of V
    in the E@V matmul.
  - Everything is computed transposed (q^T, k^T laid out [d, n]) so no
    on-device transposes are needed anywhere.

Schedule notes (final, ~208us HW at nominal clock vs 243us baseline):
  - Steady state is jointly paced by ACT (exp, ~1.0-1.1us/jt) and PE (dots
    pair row-tiled concurrent + 2 E@V matmuls ~0.75us/jt + woven filler).
  - Dots are emitted in 2-jt blocks (4 matmuls back-to-back) so each pair's
    row-disjoint LDWEIGHTS hides behind the other pair's matmuls; E@V trails
    by two periods.
  - Pre-phase is only Q(pair0,ic0) + K(pair0, j-tile 0); everything else
    (V per-jt JIT, K remainder mid-block-0, K/Q for later chunks, outproj)
    weaves into chunk periods as micro-op thunks at <=2 matmuls/period so an
    insert never stalls the exp pipeline. K(pair1, ic 2-3) JIT-weave into
    chunk (1,0) itself (needed only from its period 8), filling its idle.
  - Normalization (per-head 1/Z must precede the out-projection's head sum):
    E@V drains + reciprocal_approx_fast on the Z rows stay on the DVE FIFO
    (recip from SBUF -- from PSUM it returns garbage on HW), then one gpsimd
    partition-broadcast for both heads, muls; head B crosses partitions via
    a small DMA hop (DVE lanes are partition-locked; gpsimd cross-partition
    tensor ops are sim-only).
  - outproj(ic) units weave 1/period into chunk (1,ic+1) from period 7 (the
    norm chain takes ~6-7us); the last chunk's norm+outproj tail is split
    into two i-halves so half 1's chain overlaps half 0's outproj, with
    keep-warm matmuls hooked on chain products against HAM re-throttle.
"""

import os
import sys

import numpy as np

sys.path.insert(0, "/opt/trn_rl_repo")

import ml_dtypes

B, N, DIM = 2, 2048, 1024
HEADS, DHEAD = 16, 64
HG = 4  # heads per core
SCALE = DHEAD**-0.5
NCORES = 8
P = 128
NI = 512  # i-chunk (matmul moving free dim)
NIC = N // NI  # 4 i-chunks
NJT = N // P  # 16 j tiles
CT = DIM // P  # 8 contraction tiles

LAST_RESULTS = None
_CACHED_NC = None


def _ensure_profile_hook():
    """Provide antenv.axon_hooks (absent in this image) so that
    run_bass_kernel_spmd(trace=True) can NTFF-profile through axon."""
    import contextlib
    import ctypes
    import types

    try:
        import antenv.axon_hooks  # noqa: F401

        return
    except ImportError:
        pass
    if "antenv.axon_hooks" in sys.modules:
        return
    mod = types.ModuleType("antenv.axon_hooks")
    state = {"hook": None}
    mod.set_axon_ntff_profile_hook = lambda h: state.__setitem__("hook", h)
    mod.get_axon_ntff_profile_hook = lambda: state["hook"]
    sys.modules["antenv.axon_hooks"] = mod
    try:
        import antenv

        antenv.axon_hooks = mod
    except ImportError:
        pass

    so_path = "/opt/axon/libaxon_pjrt.so"
    if not os.path.exists(so_path):
        return
    try:
        lib = ctypes.CDLL(so_path)
    except OSError:
        return
    if not hasattr(lib, "axon_start_nrt_profile"):
        return
    lib.axon_start_nrt_profile.argtypes = [
        ctypes.POINTER(ctypes.c_int64),
        ctypes.c_size_t,
    ]
    lib.axon_start_nrt_profile.restype = ctypes.c_int64
    lib.axon_stop_nrt_profile.argtypes = [ctypes.c_char_p]
    lib.axon_stop_nrt_profile.restype = ctypes.c_int64

    @contextlib.contextmanager
    def _hook(output_dir, device_ids):
        import jax

        jax.devices()
        if device_ids:
            ids = (ctypes.c_int64 * len(device_ids))(*device_ids)
            rc = lib.axon_start_nrt_profile(ids, len(device_ids))
        else:
            rc = lib.axon_start_nrt_profile(None, 0)
        if rc != 0:
            raise RuntimeError(f"axon_start_nrt_profile rc={rc}")
        try:
            yield
        finally:
            n = lib.axon_stop_nrt_profile(str(output_dir).encode())
            print(f"ntff profile: {n} file(s) written to {output_dir}")

    mod.set_axon_ntff_profile_hook(_hook)


def _build_program():
    import concourse.tile as tile
    from concourse import bacc, mybir

    f32 = mybir.dt.float32
    bf16 = mybir.dt.bfloat16
    Exp = mybir.ActivationFunctionType.Exp

    nc = bacc.Bacc("TRN2", target_bir_lowering=False, debug=False)
    xT = nc.dram_tensor("xT", [DIM, N], bf16, kind="ExternalInput").ap()
    wq = nc.dram_tensor("wq", [DIM, HG * DHEAD], bf16, kind="ExternalInput").ap()
    wk = nc.dram_tensor("wk", [DIM, HG * DHEAD], bf16, kind="ExternalInput").ap()
    wv = nc.dram_tensor("wv", [DIM, HG * DHEAD], bf16, kind="ExternalInput").ap()
    wo = nc.dram_tensor("wo", [HG * DHEAD, DIM], bf16, kind="ExternalInput").ap()
    out = nc.dram_tensor("out", [N, DIM], bf16, kind="ExternalOutput").ap()

    with tile.TileContext(nc) as tc:
        _emit(tc, nc, mybir, out, xT, wq, wk, wv, wo, f32, bf16, Exp)
    nc.compile()
    return nc


def _emit(tc, nc, mybir, out, xT, wq, wk, wv, wo, f32, bf16, Exp):
    with (
        tc.tile_pool(name="cpool", bufs=1) as cpool,
        tc.tile_pool(name="apool", bufs=2, space="PSUM") as apool,  # proj accums
        tc.tile_pool(name="otpool", bufs=2, space="PSUM") as otpool,  # EV accums
        tc.tile_pool(name="dpool", bufs=2, space="PSUM") as dpool,  # dots
        tc.tile_pool(name="epool", bufs=2) as epool,
        tc.tile_pool(name="wpool", bufs=2) as wpool,
        tc.tile_pool(name="opool", bufs=3) as opool,
    ):
        # ---- constants. DMA order: the first K/Q projections lead, so wk/wq
        # and x^T column-chunk 0 go first; wv is needed ~6us in (V weaves into
        # chunk 0), wo much later.
        xt = cpool.tile([P, CT, N], bf16, name="xt")
        wq_sb = cpool.tile([P, CT, 256], bf16, name="wq_sb")
        wk_sb = cpool.tile([P, CT, 256], bf16, name="wk_sb")
        wv_sb = cpool.tile([P, CT, 256], bf16, name="wv_sb")
        wo_sb = cpool.tile([P, 2, DIM], bf16, name="wo_sb")

        def _xt_cc(cc, eng2):
            for t in range(CT):
                eng = (nc.sync, eng2)[t % 2]
                eng.dma_start(
                    xt[:, t, cc * NI : (cc + 1) * NI],
                    xT[t * P : (t + 1) * P, cc * NI : (cc + 1) * NI],
                )

        # descriptor generation costs ~0.6-0.8us of sequencer time per
        # dma_start, so the startup set is spread over three queues with the
        # two critical weights (wk, wq) leading on their own queues.
        nc.gpsimd.dma_start(wk_sb[:], wk.rearrange("(t p) m -> p t m", p=P))
        nc.scalar.dma_start(wq_sb[:], wq.rearrange("(t p) m -> p t m", p=P))
        engs = [nc.sync, nc.scalar, nc.gpsimd]
        for t in range(CT):
            engs[t % 3].dma_start(
                xt[:, t, 0:NI], xT[t * P : (t + 1) * P, 0:NI]
            )
        nc.gpsimd.dma_start(wv_sb[:], wv.rearrange("(t p) m -> p t m", p=P))
        for cc in range(1, 4):
            _xt_cc(cc, nc.gpsimd)
        nc.gpsimd.dma_start(wo_sb[:], wo.rearrange("(t p) m -> p t m", p=P))

        # Q^T, K^T [256, N] as 2 partition-tiles; V padded to 65 cols per
        # head: [v(64) | ones(1)] so the E@V matmul's 65th output row is Z.
        QT = cpool.tile([P, 2, N], bf16, name="QT")
        KT = cpool.tile([P, 2, N], bf16, name="KT")
        Vo = cpool.tile([P, NJT, HG * 65], bf16, name="Vo")
        Vo_heads = Vo.rearrange("p j (h c) -> p j h c", c=65)
        nc.vector.memset(Vo_heads[:, :, :, 64:65], 1.0)

        # ---- emission helpers ----
        def emit_proj_q(pt, ic):
            q_ps = apool.tile([P, NI], f32, tag="acc", name="q_ps")
            for ct in range(CT):
                nc.tensor.matmul(
                    q_ps,
                    lhsT=wq_sb[:, ct, pt * P : (pt + 1) * P],
                    rhs=xt[:, ct, ic * NI : (ic + 1) * NI],
                    start=(ct == 0),
                    stop=(ct == CT - 1),
                )
            nc.vector.tensor_copy(out=QT[:, pt, ic * NI : (ic + 1) * NI], in_=q_ps)

        def emit_proj_k(pt, ic, c0=0, c1=NI):
            k_ps = apool.tile([P, c1 - c0], f32, tag="acc", name="k_ps")
            for ct in range(CT):
                nc.tensor.matmul(
                    k_ps,
                    lhsT=wk_sb[:, ct, pt * P : (pt + 1) * P],
                    rhs=xt[:, ct, ic * NI + c0 : ic * NI + c1],
                    start=(ct == 0),
                    stop=(ct == CT - 1),
                )
            nc.vector.tensor_copy(
                out=KT[:, pt, ic * NI + c0 : ic * NI + c1], in_=k_ps
            )

        def emit_proj_v(jt):
            v_ps = apool.tile([P, 256], f32, tag="acc", name="v_ps")
            for ct in range(CT):
                nc.tensor.matmul(
                    v_ps,
                    lhsT=xt[:, ct, jt * P : (jt + 1) * P],
                    rhs=wv_sb[:, ct, :],
                    start=(ct == 0),
                    stop=(ct == CT - 1),
                )
            nc.vector.tensor_copy(
                out=Vo_heads[:, jt, :, 0:64],
                in_=v_ps.rearrange("p (h c) -> p h c", h=HG),
            )

        def ops_proj(which, pt, ic):
            """A Q/K projection group as 9 micro-op thunks (8 MMs + drain) so
            the weave can insert <=1-2 matmuls per period instead of a 2.4us
            8-MM block that would stall the exp pipeline."""
            w_sb, dst = (wq_sb, QT) if which == "q" else (wk_sb, KT)
            state = {}

            def mk_mm(ct):
                def f():
                    if ct == 0:
                        state["ps"] = apool.tile(
                            [P, NI], f32, tag="acc", name=f"{which}_ps"
                        )
                    nc.tensor.matmul(
                        state["ps"],
                        lhsT=w_sb[:, ct, pt * P : (pt + 1) * P],
                        rhs=xt[:, ct, ic * NI : (ic + 1) * NI],
                        start=(ct == 0),
                        stop=(ct == CT - 1),
                    )

                return f

            def drain():
                nc.vector.tensor_copy(
                    out=dst[:, pt, ic * NI : (ic + 1) * NI], in_=state["ps"]
                )

            return [mk_mm(ct) for ct in range(CT)] + [drain]

        onp_tiles = {}

        def emit_norm_front(pt, ic, otA, otB, late_drain=False):
            """Drain E@V out of PSUM (frees the accum banks for the next
            chunk) and compute 1/Z straight off the PSUM Z rows. Everything
            stays on the DVE FIFO -- no DMA hops until the broadcast.
            late_drain (tail only): run the Z chain first and the block
            drains after, shortening the critical path to the broadcast
            (mid-phase the drains must lead so the next chunk's E@V can
            reuse the PSUM banks)."""
            zf0 = wpool.tile([1, 2 * NI], f32, tag="zf0", name="zf0", bufs=2)
            tA = wpool.tile([64, NI], bf16, tag="tAB", name="tA", bufs=4)
            tB = wpool.tile([64, NI], bf16, tag="tAB", name="tB", bufs=4)
            if not late_drain:
                nc.vector.tensor_copy(out=tA, in_=otA[0:64, :])
            nc.vector.tensor_copy(out=zf0[0:1, 0:NI], in_=otA[64:65, :])
            if not late_drain:
                nc.vector.tensor_copy(out=tB, in_=otB[0:64, :])
            nc.vector.tensor_copy(out=zf0[0:1, NI : 2 * NI], in_=otB[64:65, :])
            zf = wpool.tile([1, 2 * NI], f32, tag="zf", name="zf", bufs=2)
            nc.vector.reciprocal_approx_fast(out=zf, in_=zf0)
            zi0 = wpool.tile([1, 2 * NI], bf16, tag="zi0", name="zi0", bufs=2)
            nc.vector.tensor_copy(out=zi0, in_=zf)
            if late_drain:
                nc.vector.tensor_copy(out=tA, in_=otA[0:64, :])
                nc.vector.tensor_copy(out=tB, in_=otB[0:64, :])
            return (pt, ic, zi0, tA, tB)

        def emit_norm_back(st):
            """Broadcast 1/Z across partitions (both heads in one go),
            normalize; head B lands in onp[64:128] via a gpsimd multiply
            (the gpsimd DSPs can write a different partition range than
            they read, unlike the DVE)."""
            pt, ic, zi0, tA, tB = st
            onp = wpool.tile([P, NI], bf16, tag=f"onp{pt}_{ic}", name="onp")
            zb2 = wpool.tile([64, 2 * NI], bf16, tag="zbb", name="zb2", bufs=2)
            nc.gpsimd.partition_broadcast(zb2[:], zi0[0:1, :])
            # normalize B first: the cross-partition hop (rows 0-63 -> 64-127,
            # DVE lanes are physically partition-locked so this needs a DMA)
            # is the longest-latency edge of the chain
            nbuf = wpool.tile([64, NI], bf16, tag="nbuf", name="nbuf")
            nc.vector.tensor_mul(out=nbuf, in0=tB[0:64, :], in1=zb2[:, NI : 2 * NI])
            nc.gpsimd.dma_start(onp[64:128, :], nbuf)
            nc.vector.tensor_mul(out=onp[0:64, :], in0=tA[0:64, :], in1=zb2[:, 0:NI])
            onp_tiles[(pt, ic)] = onp
            return zb2

        def emit_outproj_unit(ic, it_in, ec):
            """One outproj PSUM group: out[i-tile, e-half] over both pairs."""
            o_ps = apool.tile([P, NI], f32, tag="acc", name="o_ps")
            for pt in range(2):
                nc.tensor.matmul(
                    o_ps,
                    lhsT=onp_tiles[(pt, ic)][:, it_in * P : (it_in + 1) * P],
                    rhs=wo_sb[:, pt, ec * NI : (ec + 1) * NI],
                    start=(pt == 0),
                    stop=(pt == 1),
                )
            osb = opool.tile([P, NI], bf16, tag="osb", name="osb")
            nc.vector.tensor_copy(out=osb, in_=o_ps)
            it = ic * (NI // P) + it_in
            eng = (nc.sync, nc.gpsimd)[(it_in + ec) % 2]
            eng.dma_start(out[it * P : (it + 1) * P, ec * NI : (ec + 1) * NI], osb)

        # ---- chunk machinery ----
        def emit_ot(pt, jt, E, otA, otB):
            hA, hB = 2 * pt, 2 * pt + 1
            nc.tensor.matmul(
                otA,
                lhsT=Vo_heads[:, jt, hA, :],
                rhs=E[:, 0, jt, :],
                start=(jt == 0),
                stop=(jt == NJT - 1),
            )
            nc.tensor.matmul(
                otB,
                lhsT=Vo_heads[:, jt, hB, :],
                rhs=E[:, 1, jt, :],
                start=(jt == 0),
                stop=(jt == NJT - 1),
            )

        def emit_dots(pt, jt, ic):
            dAB = dpool.tile([P, 2, NI], f32, tag="dAB", name="dAB")
            nc.tensor.matmul(
                dAB[:, 0, :],
                lhsT=KT[0:64, pt, jt * P : (jt + 1) * P],
                rhs=QT[0:64, pt, ic * NI : (ic + 1) * NI],
                start=True,
                stop=True,
            )
            nc.tensor.matmul(
                dAB[:, 1, :],
                lhsT=KT[64:128, pt, jt * P : (jt + 1) * P],
                rhs=QT[64:128, pt, ic * NI : (ic + 1) * NI],
                start=True,
                stop=True,
            )
            return dAB

        def emit_chunk(pt, ic, fillers, boundary=None):
            """One (head-pair, i-chunk), in 2-jt blocks: both jt's dots pairs
            are emitted back-to-back so each pair's row-disjoint LDWEIGHTS
            hides behind the other pair's matmuls (a lone pair pays ~100ns of
            exposed weight-load per period); exps follow; E@V trails by two
            periods; fillers last. `boundary` (the previous chunk's trailing
            E@Vs + norm) is emitted after block 0's dots so the first exp of
            this chunk only waits on one dots pair, not the whole boundary.
            The final two E@V pairs are NOT emitted -- the caller gets them
            in a thunk via the return value."""
            E = epool.tile([P, 2, NJT, NI], bf16, tag="E", name="E")
            otA = otpool.tile([65, NI], f32, tag="ot", name="otA")
            otB = otpool.tile([65, NI], f32, tag="ot", name="otB")
            if boundary is not None:
                boundary()
            for b in range(NJT // 2):
                j0, j1 = 2 * b, 2 * b + 1
                d0 = emit_dots(pt, j0, ic)
                nc.scalar.activation(
                    out=E[:, :, j0, :], in_=d0[:], func=Exp, scale=SCALE
                )
                if b == 0:
                    for th in fillers.get("mid0", ()):
                        th()
                d1 = emit_dots(pt, j1, ic)
                nc.scalar.activation(
                    out=E[:, :, j1, :], in_=d1[:], func=Exp, scale=SCALE
                )
                if b > 0:
                    emit_ot(pt, j0 - 2, E, otA, otB)
                    emit_ot(pt, j0 - 1, E, otA, otB)
                for jt in (j0, j1):
                    for th in fillers.get(jt, ()):
                        th()

            def trailing_evs():
                emit_ot(pt, NJT - 2, E, otA, otB)
                emit_ot(pt, NJT - 1, E, otA, otB)

            return otA, otB, trailing_evs

        # ---- schedule ----
        # pre-phase: just enough for chunk (0,0)'s first dots pair -- Q over
        # the full i-chunk but K only for j-tile 0; the rest of K(0,0) weaves
        # in right after the first exp is underway (the whole pre-phase runs
        # at the cold 1.2GHz PE clock, so every matmul here counts double).
        emit_proj_q(0, 0)
        emit_proj_k(0, 0, 0, P)
        # V(0), V(1) fill the DMA-paced gaps of the pre-phase (their inputs
        # land by ~14us; the list scheduler interleaves them by readiness)
        emit_proj_v(0)
        emit_proj_v(1)

        def mk(fn, *a):
            return lambda: fn(*a)

        def spread(f, ops, p0, p1):
            """Distribute micro-op thunks over periods [p0, p1], preserving
            order (an accumulation group's drain must follow all its MMs)."""
            n = p1 - p0 + 1
            for i, op in enumerate(ops):
                f.setdefault(p0 + min(i * n // len(ops), n - 1), []).append(op)

        # filler schedules per chunk, keyed by period. Chunk (0,0) is
        # PE-bound by construction (V JIT + K pair-0), so its fillers stay
        # coarse; everywhere else micro-ops go in at <=2/period so a filler
        # insert never stalls the exp pipeline by more than ~0.5us.
        chunk_fillers = {}
        f00 = {}
        f00["mid0"] = [mk(emit_proj_k, 0, 0, P, NI)]
        for jt in range(2, NJT):
            f00.setdefault(jt - 1, []).append(mk(emit_proj_v, jt))
        for icK in range(1, 4):
            f00.setdefault(4 * icK - 2, []).append(mk(emit_proj_k, 0, icK))
        # whole-group here: a micro-op group left open across V(jt) allocs
        # would deadlock the 2-buffer acc pool
        f00.setdefault(13, []).append(mk(emit_proj_q, 0, 1))
        chunk_fillers[(0, 0)] = f00
        # leisure: only what chunk (1,0) needs at its START weaves into the
        # PE-bound (0,*) chunks; K(p1, 2..3) JIT-weave into (1,0) itself
        # (needed only by its dots periods 8/12), filling its idle periods.
        leisure = ops_proj("k", 1, 0) + ops_proj("k", 1, 1) + ops_proj("q", 1, 0)
        for ic in range(1, 4):
            f = {}
            if ic < 3:
                spread(f, ops_proj("q", 0, ic + 1), 0, 7)
            k = len(leisure) // 3
            spread(f, leisure[(ic - 1) * k : ic * k if ic < 3 else None], 8, 15)
            chunk_fillers[(0, ic)] = f
        # outproj(ic-1) units weave 1/period from period 7 of chunk (1,ic):
        # the pair-1 norm chain takes ~6-7us, earlier stalls the in-order PE.
        for ic in range(4):
            f = {}
            if ic == 0:
                spread(f, ops_proj("q", 1, 1), 0, 2)
                spread(f, ops_proj("k", 1, 2), 3, 6)
                spread(f, ops_proj("k", 1, 3), 7, 10)
            elif ic < 3:
                spread(f, ops_proj("q", 1, ic + 1), 0, 6)
            if ic >= 1:
                units = [
                    mk(emit_outproj_unit, ic - 1, u // 2, u % 2) for u in range(8)
                ]
                spread(f, units, 7, 14)
            chunk_fillers[(1, ic)] = f

        chunks = [(0, ic) for ic in range(NIC)] + [(1, ic) for ic in range(NIC)]
        pending = None  # (pt, ic, otA, otB) awaiting trailing EVs + norm

        def mk_boundary(p):
            def f():
                p[4]()  # trailing E@V pairs of the previous chunk
                st = emit_norm_front(*p[:4])
                emit_norm_back(st)

            return f

        for pt, ic in chunks:
            boundary = mk_boundary(pending) if pending is not None else None
            otA, otB, trail = emit_chunk(
                pt, ic, chunk_fillers[(pt, ic)], boundary=boundary
            )
            pending = (pt, ic, otA, otB, trail)
        # tail: the last chunk's norm chain + outproj are fully exposed, so
        # split them into two i-halves -- the second half's chain (DVE FIFO /
        # gpsimd) overlaps the first half's outproj matmuls. Small keep-warm
        # matmuls hooked on chain products limit HAM re-throttling.
        _, _, otA_l, otB_l, trail_l = pending
        trail_l()
        QNI = NI // 4
        warm_i = [0]

        def keep_warm(lhsT_ap, rhs_ap):
            warm_i[0] += 1
            dm = dpool.tile(
                [64, rhs_ap.free_size()], f32, tag="dAB", name=f"dm{warm_i[0]}"
            )
            nc.tensor.matmul(dm, lhsT=lhsT_ap, rhs=rhs_ap, start=True, stop=True)

        def tail_q_chain(h):
            c0, c1 = h * QNI, (h + 1) * QNI
            zf0h = wpool.tile([1, 2 * QNI], f32, tag="tzf0", name=f"zf0h{h}", bufs=4)
            nc.vector.tensor_copy(out=zf0h[0:1, 0:QNI], in_=otA_l[64:65, c0:c1])
            nc.vector.tensor_copy(out=zf0h[0:1, QNI:], in_=otB_l[64:65, c0:c1])
            zfh = wpool.tile([1, 2 * QNI], f32, tag="tzf", name=f"zfh{h}", bufs=4)
            nc.vector.reciprocal_approx_fast(out=zfh, in_=zf0h)
            zi0h = wpool.tile([1, 2 * QNI], bf16, tag="tzi0", name=f"zi0h{h}", bufs=4)
            nc.vector.tensor_copy(out=zi0h, in_=zfh)
            tAh = wpool.tile([64, QNI], bf16, tag="ttab", name=f"tAh{h}", bufs=4)
            nc.vector.tensor_copy(out=tAh, in_=otA_l[0:64, c0:c1])
            tBh = wpool.tile([64, QNI], bf16, tag="ttab", name=f"tBh{h}", bufs=4)
            nc.vector.tensor_copy(out=tBh, in_=otB_l[0:64, c0:c1])
            onph = wpool.tile([P, QNI], bf16, tag=f"onph{h}", name=f"onph{h}")
            zb2h = wpool.tile([64, 2 * QNI], bf16, tag="tzbb", name=f"zb2h{h}", bufs=4)
            nc.gpsimd.partition_broadcast(zb2h[:], zi0h[0:1, :])
            nbufh = wpool.tile([64, QNI], bf16, tag="tnbuf", name=f"nbufh{h}", bufs=4)
            nc.vector.tensor_mul(out=nbufh, in0=tBh, in1=zb2h[:, QNI:])
            (nc.gpsimd, nc.sync, nc.scalar, nc.gpsimd)[h].dma_start(
                onph[64:128, :], nbufh
            )
            nc.vector.tensor_mul(out=onph[0:64, :], in0=tAh, in1=zb2h[:, 0:QNI])
            return zi0h, onph

        def tail_q_outproj(h, onph):
            for ec in range(2):
                o_ps = apool.tile([P, NI], f32, tag="acc", name="o_ps")
                nc.tensor.matmul(
                    o_ps,
                    lhsT=onp_tiles[(0, 3)][:, h * QNI : (h + 1) * QNI],
                    rhs=wo_sb[:, 0, ec * NI : (ec + 1) * NI],
                    start=True,
                    stop=False,
                )
                nc.tensor.matmul(
                    o_ps,
                    lhsT=onph[:],
                    rhs=wo_sb[:, 1, ec * NI : (ec + 1) * NI],
                    start=False,
                    stop=True,
                )
                osb = opool.tile([P, NI], bf16, tag="osb", name="osb")
                nc.vector.tensor_copy(out=osb, in_=o_ps)
                it = 12 + h
                eng = (nc.sync, nc.gpsimd, nc.scalar)[(2 * h + ec) % 3]
                eng.dma_start(
                    out[it * P : (it + 1) * P, ec * NI : (ec + 1) * NI], osb
                )

        onps = []
        for h in range(4):
            zi0h, onph = tail_q_chain(h)
            keep_warm(KT[0:1, 0, 0:64], zi0h[0:1, 0:QNI])
            onps.append(onph)
            if h >= 1:
                tail_q_outproj(h - 1, onps[h - 1])
        tail_q_outproj(3, onps[3])


def _get_program():
    global _CACHED_NC
    if _CACHED_NC is None:
        _CACHED_NC = _build_program()
    return _CACHED_NC


def _shard_inputs(x, Wq, Wkv, Wo):
    bf = ml_dtypes.bfloat16
    xTs = [np.ascontiguousarray(x[b].T).astype(bf) for b in range(B)]
    wqs, wks, wvs, wos = [], [], [], []
    for hg in range(HG):
        c0 = hg * HG * DHEAD
        c1 = c0 + HG * DHEAD
        wqs.append(np.ascontiguousarray(Wq[:, c0:c1]).astype(bf))
        wks.append(np.ascontiguousarray(Wkv[:, c0:c1]).astype(bf))
        wvs.append(np.ascontiguousarray(Wkv[:, DIM + c0 : DIM + c1]).astype(bf))
        wos.append(np.ascontiguousarray(Wo[c0:c1, :]).astype(bf))
    in_maps = []
    for core in range(NCORES):
        b, hg = core // HG, core % HG
        in_maps.append(
            {
                "xT": xTs[b],
                "wq": wqs[hg],
                "wk": wks[hg],
                "wv": wvs[hg],
                "wo": wos[hg],
            }
        )
    return in_maps


def kernel(x, similarity, Wq, Wkv, Wo, bo):
    global LAST_RESULTS
    _ensure_profile_hook()
    import concourse.bass_utils as _bu
    from concourse.bass_utils import run_bass_kernel_spmd

    # keep trace artifacts local if profiling is ever enabled (no S3 here)
    _bu.upload_artifacts = lambda tmpdir: tmpdir

    x = np.asarray(x, dtype=np.float32)
    Wq = np.asarray(Wq, dtype=np.float32)
    Wkv = np.asarray(Wkv, dtype=np.float32)
    Wo = np.asarray(Wo, dtype=np.float32)
    bo = np.asarray(bo, dtype=np.float32)

    nc = _get_program()
    in_maps = _shard_inputs(x, Wq, Wkv, Wo)
    res = run_bass_kernel_spmd(nc, in_maps, list(range(NCORES)))
    LAST_RESULTS = res
    outs = [res.results[i]["out"] for i in range(NCORES)]
    full = np.empty((B, N, DIM), dtype=np.float32)
    for b in range(B):
        acc = outs[4 * b].astype(np.float32).copy()
        for hg in range(1, HG):
            acc += outs[4 * b + hg]
        full[b] = acc + bo[None, :]
    return full


def _sim_check():
    """Simulate core 0 on CoreSim and compare against numpy reference."""
    from concourse.bass_interp import CoreSim

    rng = np.random.default_rng(0)
    x = rng.standard_normal((B, N, DIM), dtype=np.float32)
    Wq = (rng.standard_normal((DIM, DIM), dtype=np.float32) * DIM**-0.5).astype(
        np.float32
    )
    Wkv = (
        rng.standard_normal((DIM, 2 * DIM), dtype=np.float32) * DIM**-0.5
    ).astype(np.float32)
    Wo = (rng.standard_normal((DIM, DIM), dtype=np.float32) * DIM**-0.5).astype(
        np.float32
    )

    nc = _get_program()
    in_maps = _shard_inputs(x, Wq, Wkv, Wo)
    core = 0
    sim = CoreSim(nc)
    for name, arr in in_maps[core].items():
        sim.tensor(name)[:] = arr
    sim.simulate()
    got = np.array(sim.tensor("out"))

    # numpy reference for core 0's partial (batch 0, heads 0-3), fp32 exact
    b, hg = 0, 0
    xb = x[b]
    q = xb @ Wq[:, hg * 256 : hg * 256 + 256]
    k = xb @ Wkv[:, hg * 256 : hg * 256 + 256]
    v = xb @ Wkv[:, DIM + hg * 256 : DIM + hg * 256 + 256]
    partial = np.zeros((N, DIM), dtype=np.float32)
    for h in range(HG):
        qh = q[:, h * 64 : h * 64 + 64]
        kh = k[:, h * 64 : h * 64 + 64]
        vh = v[:, h * 64 : h * 64 + 64]
        dots = (qh @ kh.T) * SCALE
        dots -= dots.max(axis=-1, keepdims=True)
        e = np.exp(dots)
        attn = e / e.sum(axis=-1, keepdims=True)
        partial += (attn @ vh) @ Wo[hg * 256 + h * 64 : hg * 256 + h * 64 + 64, :]

    err = np.abs(got - partial)
    scale = np.abs(partial).max()
    print("max abs err:", err.max(), "scale:", scale, "rel:", err.max() / scale)
    return err.max() / scale


if __name__ == "__main__":
    _sim_check()


# revision 50
# speedup vs baseline: 1.0261x; 1.0261x over previous
"""Trainium2 Bass kernel for multi-head self-attention (B=2, N=2048, DIM=1024,
16 heads x 64). Sharding: core i handles batch b=i//4 and 4 heads hg=i%4
(tensor-parallel on heads: column-shard Wq/Wkv, row-shard Wo; partial outputs
summed on host).

Math notes:
  - `similarity` adds a per-query constant along the softmax axis, so softmax
    is invariant to it -> it is accepted but unused.
  - Softmax computed without max-subtraction (logits are O(10); exp is safe in
    fp32) as exp(dots)/Z with Z obtained for free as a 65th "ones" column of V
    in the E@V matmul.
  - Everything is computed transposed (q^T, k^T laid out [d, n]) so no
    on-device transposes are needed anywhere.

Schedule notes (final, ~208us HW at nominal clock vs 243us baseline):
  - Steady state is jointly paced by ACT (exp, ~1.0-1.1us/jt) and PE (dots
    pair row-tiled concurrent + 2 E@V matmuls ~0.75us/jt + woven filler).
  - Dots are emitted in 2-jt blocks (4 matmuls back-to-back) so each pair's
    row-disjoint LDWEIGHTS hides behind the other pair's matmuls; E@V trails
    by two periods.
  - Pre-phase is only Q(pair0,ic0) + K(pair0, j-tile 0); everything else
    (V per-jt JIT, K remainder mid-block-0, K/Q for later chunks, outproj)
    weaves into chunk periods as micro-op thunks at <=2 matmuls/period so an
    insert never stalls the exp pipeline. K(pair1, ic 2-3) JIT-weave into
    chunk (1,0) itself (needed only from its period 8), filling its idle.
  - Normalization (per-head 1/Z must precede the out-projection's head sum):
    E@V drains + reciprocal_approx_fast on the Z rows stay on the DVE FIFO
    (recip from SBUF -- from PSUM it returns garbage on HW), then one gpsimd
    partition-broadcast for both heads, muls; head B crosses partitions via
    a small DMA hop (DVE lanes are partition-locked; gpsimd cross-partition
    tensor ops are sim-only).
  - outproj(ic) units weave 1/period into chunk (1,ic+1) from period 7 (the
    norm chain takes ~6-7us); the last chunk's norm+outproj tail is split
    into two i-halves so half 1's chain overlaps half 0's outproj, with
    keep-warm matmuls hooked on chain products against HAM re-throttle.
"""

import os
import sys

import numpy as np

sys.path.insert(0, "/opt/trn_rl_repo")

import ml_dtypes

B, N, DIM = 2, 2048, 1024
HEADS, DHEAD = 16, 64
HG = 4  # heads per core
SCALE = DHEAD**-0.5
NCORES = 8
P = 128
NI = 512  # i-chunk (matmul moving free dim)
NIC = N // NI  # 4 i-chunks
NJT = N // P  # 16 j tiles
CT = DIM // P  # 8 contraction tiles

LAST_RESULTS = None
_CACHED_NC = None


def _ensure_profile_hook():
    """Provide antenv.axon_hooks (absent in this image) so that
    run_bass_kernel_spmd(trace=True) can NTFF-profile through axon."""
    import contextlib
    import ctypes
    import types

    try:
        import antenv.axon_hooks  # noqa: F401

        return
    except ImportError:
        pass
    if "antenv.axon_hooks" in sys.modules:
        return
    mod = types.ModuleType("antenv.axon_hooks")
    state = {"hook": None}
    mod.set_axon_ntff_profile_hook = lambda h: state.__setitem__("hook", h)
    mod.get_axon_ntff_profile_hook = lambda: state["hook"]
    sys.modules["antenv.axon_hooks"] = mod
    try:
        import antenv

        antenv.axon_hooks = mod
    except ImportError:
        pass

    so_path = "/opt/axon/libaxon_pjrt.so"
    if not os.path.exists(so_path):
        return
    try:
        lib = ctypes.CDLL(so_path)
    except OSError:
        return
    if not hasattr(lib, "axon_start_nrt_profile"):
        return
    lib.axon_start_nrt_profile.argtypes = [
        ctypes.POINTER(ctypes.c_int64),
        ctypes.c_size_t,
    ]
    lib.axon_start_nrt_profile.restype = ctypes.c_int64
    lib.axon_stop_nrt_profile.argtypes = [ctypes.c_char_p]
    lib.axon_stop_nrt_profile.restype = ctypes.c_int64

    @contextlib.contextmanager
    def _hook(output_dir, device_ids):
        import jax

        jax.devices()
        if device_ids:
            ids = (ctypes.c_int64 * len(device_ids))(*device_ids)
            rc = lib.axon_start_nrt_profile(ids, len(device_ids))
        else:
            rc = lib.axon_start_nrt_profile(None, 0)
        if rc != 0:
            raise RuntimeError(f"axon_start_nrt_profile rc={rc}")
        try:
            yield
        finally:
            n = lib.axon_stop_nrt_profile(str(output_dir).encode())
            print(f"ntff profile: {n} file(s) written to {output_dir}")

    mod.set_axon_ntff_profile_hook(_hook)


def _build_program():
    import concourse.tile as tile
    from concourse import bacc, mybir

    f32 = mybir.dt.float32
    bf16 = mybir.dt.bfloat16
    Exp = mybir.ActivationFunctionType.Exp

    nc = bacc.Bacc("TRN2", target_bir_lowering=False, debug=False)
    xT = nc.dram_tensor("xT", [DIM, N], bf16, kind="ExternalInput").ap()
    wq = nc.dram_tensor("wq", [DIM, HG * DHEAD], bf16, kind="ExternalInput").ap()
    wk = nc.dram_tensor("wk", [DIM, HG * DHEAD], bf16, kind="ExternalInput").ap()
    wv = nc.dram_tensor("wv", [DIM, HG * DHEAD], bf16, kind="ExternalInput").ap()
    wo = nc.dram_tensor("wo", [HG * DHEAD, DIM], bf16, kind="ExternalInput").ap()
    out = nc.dram_tensor("out", [N, DIM], bf16, kind="ExternalOutput").ap()

    with tile.TileContext(nc) as tc:
        _emit(tc, nc, mybir, out, xT, wq, wk, wv, wo, f32, bf16, Exp)
    nc.compile()
    return nc


def _emit(tc, nc, mybir, out, xT, wq, wk, wv, wo, f32, bf16, Exp):
    with (
        tc.tile_pool(name="cpool", bufs=1) as cpool,
        tc.tile_pool(name="apool", bufs=2, space="PSUM") as apool,  # proj accums
        tc.tile_pool(name="otpool", bufs=2, space="PSUM") as otpool,  # EV accums
        tc.tile_pool(name="dpool", bufs=2, space="PSUM") as dpool,  # dots
        tc.tile_pool(name="epool", bufs=2) as epool,
        tc.tile_pool(name="wpool", bufs=2) as wpool,
        tc.tile_pool(name="opool", bufs=3) as opool,
    ):
        # ---- constants. DMA order: the first K/Q projections lead, so wk/wq
        # and x^T column-chunk 0 go first; wv is needed ~6us in (V weaves into
        # chunk 0), wo much later.
        xt = cpool.tile([P, CT, N], bf16, name="xt")
        wq_sb = cpool.tile([P, CT, 256], bf16, name="wq_sb")
        wk_sb = cpool.tile([P, CT, 256], bf16, name="wk_sb")
        wv_sb = cpool.tile([P, CT, 256], bf16, name="wv_sb")
        wo_sb = cpool.tile([P, 2, DIM], bf16, name="wo_sb")

        def _xt_cc(cc, eng2):
            for t in range(CT):
                eng = (nc.sync, eng2)[t % 2]
                eng.dma_start(
                    xt[:, t, cc * NI : (cc + 1) * NI],
                    xT[t * P : (t + 1) * P, cc * NI : (cc + 1) * NI],
                )

        # descriptor generation costs ~0.6-0.8us of sequencer time per
        # dma_start, so the startup set is spread over three queues with the
        # two critical weights (wk, wq) leading on their own queues.
        nc.gpsimd.dma_start(wk_sb[:], wk.rearrange("(t p) m -> p t m", p=P))
        nc.scalar.dma_start(wq_sb[:], wq.rearrange("(t p) m -> p t m", p=P))
        engs = [nc.sync, nc.scalar, nc.gpsimd]
        for t in range(CT):
            engs[t % 3].dma_start(
                xt[:, t, 0:NI], xT[t * P : (t + 1) * P, 0:NI]
            )
        nc.gpsimd.dma_start(wv_sb[:], wv.rearrange("(t p) m -> p t m", p=P))
        for cc in range(1, 4):
            _xt_cc(cc, nc.gpsimd)
        nc.gpsimd.dma_start(wo_sb[:], wo.rearrange("(t p) m -> p t m", p=P))

        # Q^T, K^T [256, N] as 2 partition-tiles; V padded to 65 cols per
        # head: [v(64) | ones(1)] so the E@V matmul's 65th output row is Z.
        QT = cpool.tile([P, 2, N], bf16, name="QT")
        KT = cpool.tile([P, 2, N], bf16, name="KT")
        Vo = cpool.tile([P, NJT, HG * 65], bf16, name="Vo")
        Vo_heads = Vo.rearrange("p j (h c) -> p j h c", c=65)
        nc.vector.memset(Vo_heads[:, :, :, 64:65], 1.0)

        # ---- emission helpers ----
        def emit_proj_q(pt, ic):
            q_ps = apool.tile([P, NI], f32, tag="acc", name="q_ps")
            for ct in range(CT):
                nc.tensor.matmul(
                    q_ps,
                    lhsT=wq_sb[:, ct, pt * P : (pt + 1) * P],
                    rhs=xt[:, ct, ic * NI : (ic + 1) * NI],
                    start=(ct == 0),
                    stop=(ct == CT - 1),
                )
            nc.vector.tensor_copy(out=QT[:, pt, ic * NI : (ic + 1) * NI], in_=q_ps)

        def emit_proj_k(pt, ic, c0=0, c1=NI):
            k_ps = apool.tile([P, c1 - c0], f32, tag="acc", name="k_ps")
            for ct in range(CT):
                nc.tensor.matmul(
                    k_ps,
                    lhsT=wk_sb[:, ct, pt * P : (pt + 1) * P],
                    rhs=xt[:, ct, ic * NI + c0 : ic * NI + c1],
                    start=(ct == 0),
                    stop=(ct == CT - 1),
                )
            nc.vector.tensor_copy(
                out=KT[:, pt, ic * NI + c0 : ic * NI + c1], in_=k_ps
            )

        def emit_proj_v(jt):
            v_ps = apool.tile([P, 256], f32, tag="acc", name="v_ps")
            for ct in range(CT):
                nc.tensor.matmul(
                    v_ps,
                    lhsT=xt[:, ct, jt * P : (jt + 1) * P],
                    rhs=wv_sb[:, ct, :],
                    start=(ct == 0),
                    stop=(ct == CT - 1),
                )
            nc.vector.tensor_copy(
                out=Vo_heads[:, jt, :, 0:64],
                in_=v_ps.rearrange("p (h c) -> p h c", h=HG),
            )

        def ops_proj(which, pt, ic):
            """A Q/K projection group as 9 micro-op thunks (8 MMs + drain) so
            the weave can insert <=1-2 matmuls per period instead of a 2.4us
            8-MM block that would stall the exp pipeline."""
            w_sb, dst = (wq_sb, QT) if which == "q" else (wk_sb, KT)
            state = {}

            def mk_mm(ct):
                def f():
                    if ct == 0:
                        state["ps"] = apool.tile(
                            [P, NI], f32, tag="acc", name=f"{which}_ps"
                        )
                    nc.tensor.matmul(
                        state["ps"],
                        lhsT=w_sb[:, ct, pt * P : (pt + 1) * P],
                        rhs=xt[:, ct, ic * NI : (ic + 1) * NI],
                        start=(ct == 0),
                        stop=(ct == CT - 1),
                    )

                return f

            def drain():
                nc.vector.tensor_copy(
                    out=dst[:, pt, ic * NI : (ic + 1) * NI], in_=state["ps"]
                )

            return [mk_mm(ct) for ct in range(CT)] + [drain]

        onp_tiles = {}

        def emit_norm_front(pt, ic, otA, otB, late_drain=False):
            """Drain E@V out of PSUM (frees the accum banks for the next
            chunk) and compute 1/Z straight off the PSUM Z rows. Everything
            stays on the DVE FIFO -- no DMA hops until the broadcast.
            late_drain (tail only): run the Z chain first and the block
            drains after, shortening the critical path to the broadcast
            (mid-phase the drains must lead so the next chunk's E@V can
            reuse the PSUM banks)."""
            zf0 = wpool.tile([1, 2 * NI], f32, tag="zf0", name="zf0", bufs=2)
            tA = wpool.tile([64, NI], bf16, tag="tAB", name="tA", bufs=4)
            tB = wpool.tile([64, NI], bf16, tag="tAB", name="tB", bufs=4)
            if not late_drain:
                nc.vector.tensor_copy(out=tA, in_=otA[0:64, :])
            nc.vector.tensor_copy(out=zf0[0:1, 0:NI], in_=otA[64:65, :])
            if not late_drain:
                nc.vector.tensor_copy(out=tB, in_=otB[0:64, :])
            nc.vector.tensor_copy(out=zf0[0:1, NI : 2 * NI], in_=otB[64:65, :])
            zf = wpool.tile([1, 2 * NI], f32, tag="zf", name="zf", bufs=2)
            nc.vector.reciprocal_approx_fast(out=zf, in_=zf0)
            zi0 = wpool.tile([1, 2 * NI], bf16, tag="zi0", name="zi0", bufs=2)
            nc.vector.tensor_copy(out=zi0, in_=zf)
            if late_drain:
                nc.vector.tensor_copy(out=tA, in_=otA[0:64, :])
                nc.vector.tensor_copy(out=tB, in_=otB[0:64, :])
            return (pt, ic, zi0, tA, tB)

        def emit_norm_back(st):
            """Broadcast 1/Z across partitions (both heads in one go),
            normalize; head B lands in onp[64:128] via a gpsimd multiply
            (the gpsimd DSPs can write a different partition range than
            they read, unlike the DVE)."""
            pt, ic, zi0, tA, tB = st
            onp = wpool.tile([P, NI], bf16, tag=f"onp{pt}_{ic}", name="onp")
            zb2 = wpool.tile([64, 2 * NI], bf16, tag="zbb", name="zb2", bufs=2)
            nc.gpsimd.partition_broadcast(zb2[:], zi0[0:1, :])
            # normalize B first: the cross-partition hop (rows 0-63 -> 64-127,
            # DVE lanes are physically partition-locked so this needs a DMA)
            # is the longest-latency edge of the chain
            nbuf = wpool.tile([64, NI], bf16, tag="nbuf", name="nbuf")
            nc.vector.tensor_mul(out=nbuf, in0=tB[0:64, :], in1=zb2[:, NI : 2 * NI])
            nc.gpsimd.dma_start(onp[64:128, :], nbuf)
            nc.vector.tensor_mul(out=onp[0:64, :], in0=tA[0:64, :], in1=zb2[:, 0:NI])
            onp_tiles[(pt, ic)] = onp
            return zb2

        def emit_outproj_unit(ic, it_in, ec):
            """One outproj PSUM group: out[i-tile, e-half] over both pairs."""
            o_ps = apool.tile([P, NI], f32, tag="acc", name="o_ps")
            for pt in range(2):
                nc.tensor.matmul(
                    o_ps,
                    lhsT=onp_tiles[(pt, ic)][:, it_in * P : (it_in + 1) * P],
                    rhs=wo_sb[:, pt, ec * NI : (ec + 1) * NI],
                    start=(pt == 0),
                    stop=(pt == 1),
                )
            osb = opool.tile([P, NI], bf16, tag="osb", name="osb")
            nc.vector.tensor_copy(out=osb, in_=o_ps)
            it = ic * (NI // P) + it_in
            eng = (nc.sync, nc.gpsimd)[(it_in + ec) % 2]
            eng.dma_start(out[it * P : (it + 1) * P, ec * NI : (ec + 1) * NI], osb)

        # ---- chunk machinery ----
        def emit_ot(pt, jt, E, otA, otB):
            hA, hB = 2 * pt, 2 * pt + 1
            nc.tensor.matmul(
                otA,
                lhsT=Vo_heads[:, jt, hA, :],
                rhs=E[:, 0, jt, :],
                start=(jt == 0),
                stop=(jt == NJT - 1),
            )
            nc.tensor.matmul(
                otB,
                lhsT=Vo_heads[:, jt, hB, :],
                rhs=E[:, 1, jt, :],
                start=(jt == 0),
                stop=(jt == NJT - 1),
            )

        def emit_dots(pt, jt, ic):
            dAB = dpool.tile([P, 2, NI], f32, tag="dAB", name="dAB")
            nc.tensor.matmul(
                dAB[:, 0, :],
                lhsT=KT[0:64, pt, jt * P : (jt + 1) * P],
                rhs=QT[0:64, pt, ic * NI : (ic + 1) * NI],
                start=True,
                stop=True,
            )
            nc.tensor.matmul(
                dAB[:, 1, :],
                lhsT=KT[64:128, pt, jt * P : (jt + 1) * P],
                rhs=QT[64:128, pt, ic * NI : (ic + 1) * NI],
                start=True,
                stop=True,
            )
            return dAB

        def emit_chunk(pt, ic, fillers, boundary=None):
            """One (head-pair, i-chunk), in 2-jt blocks: both jt's dots pairs
            are emitted back-to-back so each pair's row-disjoint LDWEIGHTS
            hides behind the other pair's matmuls (a lone pair pays ~100ns of
            exposed weight-load per period); exps follow; E@V trails by two
            periods; fillers last. `boundary` (the previous chunk's trailing
            E@Vs + norm) is emitted after block 0's dots so the first exp of
            this chunk only waits on one dots pair, not the whole boundary.
            The final two E@V pairs are NOT emitted -- the caller gets them
            in a thunk via the return value."""
            E = epool.tile([P, 2, NJT, NI], bf16, tag="E", name="E")
            otA = otpool.tile([65, NI], f32, tag="ot", name="otA")
            otB = otpool.tile([65, NI], f32, tag="ot", name="otB")
            if boundary is not None:
                boundary()
            for b in range(NJT // 2):
                j0, j1 = 2 * b, 2 * b + 1
                d0 = emit_dots(pt, j0, ic)
                nc.scalar.activation(
                    out=E[:, :, j0, :], in_=d0[:], func=Exp, scale=SCALE
                )
                if b == 0:
                    for th in fillers.get("mid0", ()):
                        th()
                d1 = emit_dots(pt, j1, ic)
                nc.scalar.activation(
                    out=E[:, :, j1, :], in_=d1[:], func=Exp, scale=SCALE
                )
                if b > 0:
                    emit_ot(pt, j0 - 2, E, otA, otB)
                    emit_ot(pt, j0 - 1, E, otA, otB)
                for jt in (j0, j1):
                    for th in fillers.get(jt, ()):
                        th()

            def trailing_evs():
                emit_ot(pt, NJT - 2, E, otA, otB)
                emit_ot(pt, NJT - 1, E, otA, otB)

            return otA, otB, trailing_evs

        # ---- schedule ----
        # pre-phase: just enough for chunk (0,0)'s first dots pair -- Q over
        # the full i-chunk but K only for j-tile 0; the rest of K(0,0) weaves
        # in right after the first exp is underway (the whole pre-phase runs
        # at the cold 1.2GHz PE clock, so every matmul here counts double).
        emit_proj_q(0, 0)
        emit_proj_k(0, 0, 0, P)
        # V(0), V(1) fill the DMA-paced gaps of the pre-phase (their inputs
        # land by ~14us; the list scheduler interleaves them by readiness)
        emit_proj_v(0)
        emit_proj_v(1)

        def mk(fn, *a):
            return lambda: fn(*a)

        def spread(f, ops, p0, p1):
            """Distribute micro-op thunks over periods [p0, p1], preserving
            order (an accumulation group's drain must follow all its MMs)."""
            n = p1 - p0 + 1
            for i, op in enumerate(ops):
                f.setdefault(p0 + min(i * n // len(ops), n - 1), []).append(op)

        # filler schedules per chunk, keyed by period. Chunk (0,0) is
        # PE-bound by construction (V JIT + K pair-0), so its fillers stay
        # coarse; everywhere else micro-ops go in at <=2/period so a filler
        # insert never stalls the exp pipeline by more than ~0.5us.
        chunk_fillers = {}
        f00 = {}
        f00["mid0"] = [mk(emit_proj_k, 0, 0, P, NI)]
        for jt in range(2, NJT):
            f00.setdefault(jt - 1, []).append(mk(emit_proj_v, jt))
        for icK in range(1, 4):
            f00.setdefault(4 * icK - 2, []).append(mk(emit_proj_k, 0, icK))
        # whole-group here: a micro-op group left open across V(jt) allocs
        # would deadlock the 2-buffer acc pool
        f00.setdefault(13, []).append(mk(emit_proj_q, 0, 1))
        chunk_fillers[(0, 0)] = f00
        # leisure: only what chunk (1,0) needs at its START weaves into the
        # PE-bound (0,*) chunks; K(p1, 2..3) JIT-weave into (1,0) itself
        # (needed only by its dots periods 8/12), filling its idle periods.
        leisure = ops_proj("k", 1, 0) + ops_proj("k", 1, 1) + ops_proj("q", 1, 0)
        for ic in range(1, 4):
            f = {}
            if ic < 3:
                spread(f, ops_proj("q", 0, ic + 1), 0, 7)
            k = len(leisure) // 3
            spread(f, leisure[(ic - 1) * k : ic * k if ic < 3 else None], 8, 15)
            chunk_fillers[(0, ic)] = f
        # outproj(ic-1) units weave 1/period from period 7 of chunk (1,ic):
        # the pair-1 norm chain takes ~6-7us, earlier stalls the in-order PE.
        for ic in range(4):
            f = {}
            if ic == 0:
                spread(f, ops_proj("q", 1, 1), 0, 2)
                spread(f, ops_proj("k", 1, 2), 3, 6)
                spread(f, ops_proj("k", 1, 3), 7, 10)
            elif ic < 3:
                spread(f, ops_proj("q", 1, ic + 1), 0, 6)
            if ic >= 1:
                units = [
                    mk(emit_outproj_unit, ic - 1, u // 2, u % 2) for u in range(8)
                ]
                spread(f, units, 7, 14)
            chunk_fillers[(1, ic)] = f

        chunks = [(0, ic) for ic in range(NIC)] + [(1, ic) for ic in range(NIC)]
        pending = None  # (pt, ic, otA, otB) awaiting trailing EVs + norm

        def mk_boundary(p):
            def f():
                p[4]()  # trailing E@V pairs of the previous chunk
                st = emit_norm_front(*p[:4])
                emit_norm_back(st)

            return f

        for pt, ic in chunks:
            boundary = mk_boundary(pending) if pending is not None else None
            otA, otB, trail = emit_chunk(
                pt, ic, chunk_fillers[(pt, ic)], boundary=boundary
            )
            pending = (pt, ic, otA, otB, trail)
        # tail: the last chunk's norm chain + outproj are fully exposed, so
        # split them into two i-halves -- the second half's chain (DVE FIFO /
        # gpsimd) overlaps the first half's outproj matmuls. Small keep-warm
        # matmuls hooked on chain products limit HAM re-throttling.
        _, _, otA_l, otB_l, trail_l = pending
        trail_l()
        HNI = NI // 2
        warm_i = [0]

        def keep_warm(lhsT_ap, rhs_ap):
            warm_i[0] += 1
            dm = dpool.tile(
                [64, rhs_ap.free_size()], f32, tag="dAB", name=f"dm{warm_i[0]}"
            )
            nc.tensor.matmul(dm, lhsT=lhsT_ap, rhs=rhs_ap, start=True, stop=True)

        def tail_half_chain(h):
            c0, c1 = h * HNI, (h + 1) * HNI
            zf0h = wpool.tile([1, NI], f32, tag="tzf0", name=f"zf0h{h}", bufs=2)
            nc.vector.tensor_copy(out=zf0h[0:1, 0:HNI], in_=otA_l[64:65, c0:c1])
            nc.vector.tensor_copy(out=zf0h[0:1, HNI:NI], in_=otB_l[64:65, c0:c1])
            zfh = wpool.tile([1, NI], f32, tag="tzf", name=f"zfh{h}", bufs=2)
            nc.vector.reciprocal_approx_fast(out=zfh, in_=zf0h)
            zi0h = wpool.tile([1, NI], bf16, tag="tzi0", name=f"zi0h{h}", bufs=2)
            nc.vector.tensor_copy(out=zi0h, in_=zfh)
            tAh = wpool.tile([64, HNI], bf16, tag="ttab", name=f"tAh{h}", bufs=4)
            nc.vector.tensor_copy(out=tAh, in_=otA_l[0:64, c0:c1])
            tBh = wpool.tile([64, HNI], bf16, tag="ttab", name=f"tBh{h}", bufs=4)
            nc.vector.tensor_copy(out=tBh, in_=otB_l[0:64, c0:c1])
            onph = wpool.tile([P, HNI], bf16, tag=f"onph{h}", name=f"onph{h}")
            zb2h = wpool.tile([64, NI], bf16, tag="tzbb", name=f"zb2h{h}", bufs=2)
            nc.gpsimd.partition_broadcast(zb2h[:], zi0h[0:1, :])
            nbufh = wpool.tile([64, HNI], bf16, tag="tnbuf", name=f"nbufh{h}", bufs=2)
            nc.vector.tensor_mul(out=nbufh, in0=tBh, in1=zb2h[:, HNI:NI])
            (nc.gpsimd, nc.sync)[h].dma_start(onph[64:128, :], nbufh)
            nc.vector.tensor_mul(out=onph[0:64, :], in0=tAh, in1=zb2h[:, 0:HNI])
            return zf0h, zi0h, zb2h, onph

        def tail_half_outproj(h, onph):
            for u in range(4):
                it2, ec = u // 2, u % 2
                o_ps = apool.tile([P, NI], f32, tag="acc", name="o_ps")
                nc.tensor.matmul(
                    o_ps,
                    lhsT=onp_tiles[(0, 3)][
                        :, h * HNI + it2 * P : h * HNI + (it2 + 1) * P
                    ],
                    rhs=wo_sb[:, 0, ec * NI : (ec + 1) * NI],
                    start=True,
                    stop=False,
                )
                nc.tensor.matmul(
                    o_ps,
                    lhsT=onph[:, it2 * P : (it2 + 1) * P],
                    rhs=wo_sb[:, 1, ec * NI : (ec + 1) * NI],
                    start=False,
                    stop=True,
                )
                osb = opool.tile([P, NI], bf16, tag="osb", name="osb")
                nc.vector.tensor_copy(out=osb, in_=o_ps)
                it = 12 + 2 * h + it2
                eng = (nc.sync, nc.gpsimd, nc.scalar)[u % 3]
                eng.dma_start(
                    out[it * P : (it + 1) * P, ec * NI : (ec + 1) * NI], osb
                )

        zf00, zi00, zb20, onp0 = tail_half_chain(0)
        keep_warm(KT[0:1, 0, 0:64], zi00[0:1, 0:NI])
        zf01, zi01, zb21, onp1 = tail_half_chain(1)
        keep_warm(KT[0:64, 0, 0:64], zb20[:, 0:NI])
        tail_half_outproj(0, onp0)
        tail_half_outproj(1, onp1)


def _get_program():
    global _CACHED_NC
    if _CACHED_NC is None:
        _CACHED_NC = _build_program()
    return _CACHED_NC


def _shard_inputs(x, Wq, Wkv, Wo):
    bf = ml_dtypes.bfloat16
    xTs = [np.ascontiguousarray(x[b].T).astype(bf) for b in range(B)]
    wqs, wks, wvs, wos = [], [], [], []
    for hg in range(HG):
        c0 = hg * HG * DHEAD
        c1 = c0 + HG * DHEAD
        wqs.append(np.ascontiguousarray(Wq[:, c0:c1]).astype(bf))
        wks.append(np.ascontiguousarray(Wkv[:, c0:c1]).astype(bf))
        wvs.append(np.ascontiguousarray(Wkv[:, DIM + c0 : DIM + c1]).astype(bf))
        wos.append(np.ascontiguousarray(Wo[c0:c1, :]).astype(bf))
    in_maps = []
    for core in range(NCORES):
        b, hg = core // HG, core % HG
        in_maps.append(
            {
                "xT": xTs[b],
                "wq": wqs[hg],
                "wk": wks[hg],
                "wv": wvs[hg],
                "wo": wos[hg],
            }
        )
    return in_maps


def kernel(x, similarity, Wq, Wkv, Wo, bo):
    global LAST_RESULTS
    _ensure_profile_hook()
    import concourse.bass_utils as _bu
    from concourse.bass_utils import run_bass_kernel_spmd

    # keep trace artifacts local if profiling is ever enabled (no S3 here)
    _bu.upload_artifacts = lambda tmpdir: tmpdir

    x = np.asarray(x, dtype=np.float32)
    Wq = np.asarray(Wq, dtype=np.float32)
    Wkv = np.asarray(Wkv, dtype=np.float32)
    Wo = np.asarray(Wo, dtype=np.float32)
    bo = np.asarray(bo, dtype=np.float32)

    nc = _get_program()
    in_maps = _shard_inputs(x, Wq, Wkv, Wo)
    res = run_bass_kernel_spmd(nc, in_maps, list(range(NCORES)))
    LAST_RESULTS = res
    outs = [res.results[i]["out"] for i in range(NCORES)]
    full = np.empty((B, N, DIM), dtype=np.float32)
    for b in range(B):
        acc = outs[4 * b].astype(np.float32).copy()
        for hg in range(1, HG):
            acc += outs[4 * b + hg]
        full[b] = acc + bo[None, :]
    return full


def _sim_check():
    """Simulate core 0 on CoreSim and compare against numpy reference."""
    from concourse.bass_interp import CoreSim

    rng = np.random.default_rng(0)
    x = rng.standard_normal((B, N, DIM), dtype=np.float32)
    Wq = (rng.standard_normal((DIM, DIM), dtype=np.float32) * DIM**-0.5).astype(
        np.float32
    )
    Wkv = (
        rng.standard_normal((DIM, 2 * DIM), dtype=np.float32) * DIM**-0.5
    ).astype(np.float32)
    Wo = (rng.standard_normal((DIM, DIM), dtype=np.float32) * DIM**-0.5).astype(
        np.float32
    )

    nc = _get_program()
    in_maps = _shard_inputs(x, Wq, Wkv, Wo)
    core = 0
    sim = CoreSim(nc)
    for name, arr in in_maps[core].items():
        sim.tensor(name)[:] = arr
    sim.simulate()
    got = np.array(sim.tensor("out"))

    # numpy reference for core 0's partial (batch 0, heads 0-3), fp32 exact
    b, hg = 0, 0
    xb = x[b]
    q = xb @ Wq[:, hg * 256 : hg * 256 + 256]
    k = xb @ Wkv[:, hg * 256 : hg * 256 + 256]
    v = xb @ Wkv[:, DIM + hg * 256 : DIM + hg * 256 + 256]
    partial = np.zeros((N, DIM), dtype=np.float32)
    for h in range(HG):
        qh = q[:, h * 64 : h * 64 + 64]
        kh = k[:, h * 64 : h * 64 + 64]
        vh = v[:, h * 64 : h * 64 + 64]
        dots = (qh @ kh.T) * SCALE
        dots -= dots.max(axis=-1, keepdims=True)
        e = np.exp(dots)
        attn = e / e.sum(axis=-1, keepdims=True)
        partial += (attn @ vh) @ Wo[hg * 256 + h * 64 : hg * 256 + h * 64 + 64, :]

    err = np.abs(got - partial)
    scale = np.abs(partial).max()
    print("max abs err:", err.max(), "scale:", scale, "rel:", err.max() / scale)
    return err.max() / scale


if __name__ == "__main__":
    _sim_check()
